# revision 1
# baseline (speedup 1.0000x reference)
"""Trainium2 Bass kernel for nn_MixedSparseSingleLayer (dense transformer layer
with LoRA adapters): RMSNorm -> QKV(+LoRA) -> RoPE -> causal attention ->
O-proj(+LoRA) -> residual -> RMSNorm -> MLP silu(up)+down (+LoRA) -> residual.

Sharding (8 NeuronCores): 2-way data parallel over batch x 4-way tensor
parallel (Megatron). Within a 4-core batch group:
  - norm1 is replicated (cheap), QKV is column-sharded so each core owns 4
    attention heads end-to-end (RoPE + causal softmax + PV).
  - attention outputs are exchanged with a single small AllToAll (1MB/peer)
    so each core then owns a 512-row slice for row-parallel O-proj + MLP.
LoRA (rank 16), biases and RMSNorm weights are folded on the host (exact
algebraic rewrites); all activations live in transposed [D, rows] layout so
matmuls run as lhsT=weight-tile / rhs=activation with N>=256 (fp32r full
rate on the PE).
"""

import numpy as np
import ml_dtypes

import concourse.bass as bass
import concourse.mybir as mybir
import concourse.tile as tile
from concourse import bacc
from concourse.bass_utils import run_bass_kernel_spmd
from concourse.masks import make_identity

f32 = mybir.dt.float32
f32r = mybir.dt.float32r
bf16 = mybir.dt.bfloat16

B, S, D, H, HD, F, R = 2, 2048, 2048, 16, 128, 8192, 16
P = 128
KD = D // P            # 16 d_model tiles
MQKV = 12              # 1536/128 output tiles of the qkv shard
NH = 4                 # heads per core
QB = S // P            # 16 q blocks per head
FT = F // P            # 64
ROWS = 512             # rows owned per core (S / 4)
RKD = KD               # k tiles over D
SCALE = 1.0 / float(np.sqrt(HD))
EPS = 1e-10

DEBUG_TAPS = False
N_CORES = 8
GROUPS = [[0, 1, 2, 3, 4, 5, 6, 7]]
RH2 = ROWS // 2        # 256: rows owned per core per batch


def build_program(single_core=False):
    nc = bacc.Bacc(
        "TRN2",
        target_bir_lowering=False,
        debug=False,
        num_devices=1 if single_core else N_CORES,
    )

    # ---- I/O ----
    xbT_in = nc.dram_tensor("xbT", [D, S], f32, kind="ExternalInput").ap()
    xrT_in = nc.dram_tensor("xrT", [D, ROWS], f32, kind="ExternalInput").ap()
    wqkv_in = nc.dram_tensor("wqkv", [MQKV, P, KD, P], f32r, kind="ExternalInput").ap()
    bqkv_in = nc.dram_tensor("bqkv", [P, MQKV], f32, kind="ExternalInput").ap()
    wo_in = nc.dram_tensor("wo", [KD, P, KD, P], f32r, kind="ExternalInput").ap()
    wup_in = nc.dram_tensor("wup", [FT, P, KD, P], f32r, kind="ExternalInput").ap()
    bup_in = nc.dram_tensor("bup", [P, FT], f32, kind="ExternalInput").ap()
    wdn_in = nc.dram_tensor("wdn", [KD, P, FT, P], bf16, kind="ExternalInput").ap()
    bdn_in = nc.dram_tensor("bdn", [P, KD], f32, kind="ExternalInput").ap()
    cosT_in = nc.dram_tensor("cosT", [P, S], f32, kind="ExternalInput").ap()
    sinTs_in = nc.dram_tensor("sinTs", [P, S], f32, kind="ExternalInput").ap()
    rotmT_in = nc.dram_tensor("rotmT", [P, P], f32r, kind="ExternalInput").ap()
    ones_in = nc.dram_tensor("ones", [P, P], f32r, kind="ExternalInput").ap()
    mask_in = nc.dram_tensor("mask", [P, 512], f32, kind="ExternalInput").ap()
    outT = nc.dram_tensor("outT", [D, ROWS], f32, kind="ExternalOutput").ap()
    dbg = {}
    if DEBUG_TAPS:
        dbg["q"] = nc.dram_tensor("dbg_q", [P, S], f32, kind="ExternalOutput").ap()
        dbg["rq"] = nc.dram_tensor("dbg_rq", [P, S], f32, kind="ExternalOutput").ap()
        dbg["oth"] = nc.dram_tensor("dbg_oth", [P, S], f32, kind="ExternalOutput").ap()
        dbg["om"] = nc.dram_tensor("dbg_om", [P, KD * ROWS], f32, kind="ExternalOutput").ap()
        dbg["x1"] = nc.dram_tensor("dbg_x1", [P, KD * ROWS], f32, kind="ExternalOutput").ap()
        dbg["xn2"] = nc.dram_tensor("dbg_xn2", [P, KD * ROWS], f32, kind="ExternalOutput").ap()
        dbg["fn"] = nc.dram_tensor("dbg_fn", [P, 8 * ROWS], bf16, kind="ExternalOutput").ap()
        dbg["rb2"] = nc.dram_tensor("dbg_rb2", [P, ROWS], f32, kind="ExternalOutput").ap()

    with tile.TileContext(nc) as tc:
        _emit(tc, nc, xbT_in, xrT_in, wqkv_in, bqkv_in, wo_in, wup_in, bup_in,
              wdn_in, bdn_in, cosT_in, sinTs_in, rotmT_in, ones_in, mask_in,
              outT, dbg, single_core)

    nc.compile()
    return nc


def _emit(tc, nc, xbT_in, xrT_in, wqkv_in, bqkv_in, wo_in, wup_in, bup_in,
          wdn_in, bdn_in, cosT_in, sinTs_in, rotmT_in, ones_in, mask_in, outT,
          dbg, single_core=False):
    from contextlib import ExitStack

    top = ExitStack()
    with top:
        consts = top.enter_context(tc.tile_pool(name="consts", bufs=1))
        ident = consts.tile([P, P], f32, tag="ident")
        make_identity(nc, ident[:])
        ones = consts.tile([P, P], f32r, tag="ones")
        nc.sync.dma_start(ones[:], ones_in)
        wmask_sb = consts.tile([P, 512], f32, tag="mask")
        nc.sync.dma_start(wmask_sb[:], mask_in)
        cosT = consts.tile([P, S], f32, tag="cosT")
        nc.sync.dma_start(cosT[:], cosT_in)
        sinTs = consts.tile([P, S], f32, tag="sinTs")
        nc.sync.dma_start(sinTs[:], sinTs_in)
        rotmT = consts.tile([P, P], f32r, tag="rotmT")
        nc.sync.dma_start(rotmT[:], rotmT_in)
        bqkv_sb = consts.tile([P, MQKV], f32, tag="bqkv")
        nc.sync.dma_start(bqkv_sb[:], bqkv_in)
        bup_sb = consts.tile([P, FT], f32, tag="bup")
        nc.sync.dma_start(bup_sb[:], bup_in)
        bdn_sb = consts.tile([P, KD], f32, tag="bdn")
        nc.sync.dma_start(bdn_sb[:], bdn_in)
        eps_sb = consts.tile([P, 1], f32, tag="eps")
        nc.vector.memset(eps_sb[:], EPS)

        # DRAM pool for the AllToAll exchange.
        # Chunk j holds this core's 4 heads (512 d) x core j's 256 rows of
        # THIS core's batch; since core j owns 256 rows of each batch, every
        # chunk of the 8-core AllToAll carries useful data and the receive
        # side needs no rank-dependent addressing.
        dram = top.enter_context(tc.tile_pool(name="a2a", bufs=1, space="DRAM"))
        a2a_in = dram.tile([N_CORES, ROWS, RH2], f32, tag="a2a_in")
        a2a_out = dram.tile([N_CORES, ROWS, RH2], f32, tag="a2a_out")

        # ================= Phase A: norm1 (replicated) + QKV shard ==========
        # qkvT holds q.T|k.T|v.T for this core's 4 heads over all S rows.
        qkv_stack = ExitStack()
        qkvp = qkv_stack.enter_context(tc.tile_pool(name="qkvT", bufs=1))
        qkvT = qkvp.tile([P, MQKV * S], f32, tag="qkvT")

        with tc.tile_pool(name="phA_sb", bufs=1) as pa, \
             tc.tile_pool(name="phA_sq", bufs=3) as sqp, \
             tc.tile_pool(name="phA_w", bufs=3) as wp, \
             tc.tile_pool(name="phA_ps", bufs=4, space="PSUM") as pps, \
             tc.tile_pool(name="phA_st", bufs=1, space="PSUM") as stps, \
             tc.tile_pool(name="phA_r", bufs=2) as rp:
            NRH = 512  # rows per chunk
            for rh in range(S // NRH):
                xn1 = pa.tile([P, KD * NRH], f32r, tag="xn1")
                for kd in range(KD):
                    nc.sync.dma_start(
                        xn1[:, kd * NRH:(kd + 1) * NRH],
                        xbT_in[kd * P:(kd + 1) * P,
                               rh * NRH:(rh + 1) * NRH].bitcast(f32r))
                # row stats: ssq[r] = sum_d x[d,r]^2  (PE ones-matmul trick)
                ssq = stps.tile([1, NRH], f32, tag="ssq")
                for kd in range(KD):
                    sq = sqp.tile([P, NRH], f32, tag="sq")
                    nc.scalar.activation(sq[:].bitcast(f32r),
                                         xn1[:, kd * NRH:(kd + 1) * NRH].bitcast(f32),
                                         mybir.ActivationFunctionType.Square)
                    nc.tensor.matmul(
                        ssq[:], ones[:, 0:1], sq[:].bitcast(f32r),
                        start=(kd == 0), stop=(kd == KD - 1))
                sqr = rp.tile([1, NRH], f32, tag="sqr")
                nc.scalar.activation(sqr[:], ssq[:],
                                     mybir.ActivationFunctionType.Sqrt,
                                     bias=eps_sb[0:1, :], scale=1.0 / D)
                rr = rp.tile([1, NRH], f32, tag="rr")
                with nc.allow_low_precision(reason="f32r rounding for PE broadcast"):
                    nc.vector.reciprocal(rr[:].bitcast(f32r), sqr[:])
                rb = stps.tile([P, NRH], f32, tag="rb")
                nc.tensor.matmul(rb[:], ones[0:1, :],
                                 rr[:].bitcast(f32r), start=True, stop=True)
                for kd in range(KD):
                    nc.vector.tensor_mul(xn1[:, kd * NRH:(kd + 1) * NRH],
                                         xn1[:, kd * NRH:(kd + 1) * NRH].bitcast(f32),
                                         rb[:])
                # QKV matmuls: head-major m order so attention can start early
                for mt in (0, 4, 8, 1, 5, 9, 2, 6, 10, 3, 7, 11):
                    wsb = wp.tile([P, KD * P], f32r, tag="wqkv")
                    nc.sync.dma_start(
                        wsb[:], wqkv_in[mt].rearrange("p k m -> p (k m)"))
                    acc = pps.tile([P, NRH], f32, tag="qkvacc")
                    for kd in range(KD):
                        nc.tensor.matmul(
                            acc[:],
                            wsb[:, kd * P:(kd + 1) * P],
                            xn1[:, kd * NRH:(kd + 1) * NRH],
                            start=(kd == 0), stop=(kd == KD - 1))
                    nc.scalar.activation(
                        qkvT[:, mt * S + rh * NRH: mt * S + rh * NRH + NRH].bitcast(f32r),
                        acc[:], mybir.ActivationFunctionType.Identity,
                        bias=bqkv_sb[:, mt:mt + 1])

        if dbg:
            nc.sync.dma_start(dbg["q"], qkvT[:, 0:S])

        # ================= Phase B: attention (4 heads) =====================
        with tc.tile_pool(name="rope", bufs=1) as ropep, \
             tc.tile_pool(name="rtmp", bufs=1) as rtmpp, \
             tc.tile_pool(name="vnat", bufs=1) as vnatp, \
             tc.tile_pool(name="prT", bufs=5) as prtp, \
             tc.tile_pool(name="lsum", bufs=4) as lp, \
             tc.tile_pool(name="rbc", bufs=2) as rbcp, \
             tc.tile_pool(name="oT", bufs=2) as otp, \
             tc.tile_pool(name="sc_ps", bufs=3, space="PSUM") as scps, \
             tc.tile_pool(name="tr_ps", bufs=1, space="PSUM") as trps, \
             tc.tile_pool(name="ov_ps", bufs=2, space="PSUM") as ovps, \
             tc.tile_pool(name="st_ps", bufs=1, space="PSUM") as stp2:
            for h in range(NH):
                q_sl = qkvT[:, h * S:(h + 1) * S]
                k_sl = qkvT[:, (NH + h) * S:(NH + h + 1) * S]
                v_sl = qkvT[:, (2 * NH + h) * S:(2 * NH + h + 1) * S]
                # RoPE on q and k. rotate_half is a cross-partition shuffle,
                # which DVE lanes cannot do, so apply it as a PE matmul with
                # a signed permutation matrix (sign of rotate_half baked in).
                rq = ropep.tile([P, S], f32r, tag="ropeq")
                rk = ropep.tile([P, S], f32r, tag="ropek")
                for src, dst in ((q_sl, rq), (k_sl, rk)):
                    tmp = rtmpp.tile([P, S], f32, tag="rtmp")
                    for c in range(S // 512):
                        rt = scps.tile([P, 512], f32, tag="scc")
                        nc.tensor.matmul(
                            rt[:],
                            rotmT[:],
                            src[:, c * 512:(c + 1) * 512].bitcast(f32r),
                            start=True, stop=True)
                        nc.vector.tensor_mul(tmp[:, c * 512:(c + 1) * 512],
                                             rt[:],
                                             sinTs[:, c * 512:(c + 1) * 512])
                    nc.vector.tensor_mul(dst[:], src[:], cosT[:])
                    nc.vector.tensor_add(dst[:], dst[:].bitcast(f32), tmp[:])
                # v in natural layout [S, HD] (PE transpose per 128-block)
                vnat = vnatp.tile([P, QB * P], f32, tag="vnat")
                for kt in range(QB):
                    tp = trps.tile([P, P], f32, tag="trp")
                    nc.tensor.transpose(tp[:], v_sl[:, kt * P:(kt + 1) * P], ident[:])
                    nc.vector.tensor_copy(vnat[:, kt * P:(kt + 1) * P].bitcast(f32r), tp[:])
                oTh = otp.tile([P, S], f32, tag="oTh")
                # q processed in 512-wide chunks; scores computed TRANSPOSED
                # (s.T[S_k, q]) so exp output is already in PV layout.
                for qc in range(S // 512):
                    opsum = ovps.tile([P, 512], f32, tag="opv")
                    lps = stp2.tile([1, 512], f32, tag="lps")
                    nkt = 4 * qc + 4
                    for kt in range(nkt):
                        scc = scps.tile([P, 512], f32, tag="scc")
                        nc.tensor.matmul(
                            scc[:],
                            rk[:, kt * P:(kt + 1) * P],
                            rq[:, qc * 512:(qc + 1) * 512],
                            start=True, stop=True)
                        lb = kt - 4 * qc
                        if lb >= 0:
                            # mask: lb fully-masked 128-blocks + triangular
                            nc.vector.tensor_add(
                                scc[:, 0:(lb + 1) * P],
                                scc[:, 0:(lb + 1) * P],
                                wmask_sb[:, (3 - lb) * P:512])
                        prT = prtp.tile([P, 512], f32, tag="prT")
                        nc.scalar.activation(
                            prT[:].bitcast(f32r), scc[:],
                            mybir.ActivationFunctionType.Exp, scale=SCALE)
                        nc.tensor.matmul(
                            lps[:], ones[:, 0:1], prT[:].bitcast(f32r),
                            start=(kt == 0), stop=(kt == nkt - 1))
                        nc.tensor.matmul(
                            opsum[:], vnat[:, kt * P:(kt + 1) * P].bitcast(f32r),
                            prT[:].bitcast(f32r),
                            start=(kt == 0), stop=(kt == nkt - 1))
                    rinv = lp.tile([1, 512], f32, tag="rinv")
                    with nc.allow_low_precision(reason="f32r rounding for PE bcast"):
                        nc.vector.reciprocal(rinv[:].bitcast(f32r), lps[:])
                    rbc = stp2.tile([P, 512], f32, tag="rbc")
                    nc.tensor.matmul(rbc[:], ones[0:1, :], rinv[:].bitcast(f32r),
                                     start=True, stop=True)
                    rbs = rbcp.tile([P, 512], f32, tag="rbs")
                    nc.vector.tensor_copy(rbs[:], rbc[:])
                    nc.vector.tensor_mul(oTh[:, qc * 512:(qc + 1) * 512],
                                         opsum[:], rbs[:])
                if dbg and h == 0:
                    nc.sync.dma_start(dbg["rq"], rq[:].bitcast(f32))
                    nc.sync.dma_start(dbg["oth"], oTh[:])
                # ship this head's output into the AllToAll staging buffer
                for j in range(N_CORES):
                    nc.sync.dma_start(
                        a2a_in[j, h * P:(h + 1) * P, :],
                        oTh[:, j * RH2:(j + 1) * RH2])
        qkv_stack.close()

        # ================= AllToAll exchange ================================
        if single_core:
            # timing-only stand-in for the collective (cost-model sim has no
            # multi-core support): move the same bytes DRAM->DRAM locally
            nc.sync.dma_start(a2a_out[:].rearrange("a r c -> (a r) c"),
                              a2a_in[:].rearrange("a r c -> (a r) c"))
        else:
            nc.gpsimd.collective_compute(
                "AllToAll", mybir.AluOpType.bypass,
                replica_groups=GROUPS,
                ins=[a2a_in[:].opt()],
                outs=[a2a_out[:].opt()],
            )

        # ================= Phase C..F: row-parallel O-proj + MLP ============
        x1_stack = ExitStack()
        x1p = x1_stack.enter_context(tc.tile_pool(name="x1T", bufs=1))
        x1T = x1p.tile([P, KD * ROWS], f32, tag="x1T")
        oview = a2a_out[:].rearrange("a r c -> (a r) c")

        with tc.tile_pool(name="phC_om", bufs=1) as omp, \
             tc.tile_pool(name="phC_xr", bufs=1) as xrp, \
             tc.tile_pool(name="phC_w", bufs=3) as wop, \
             tc.tile_pool(name="phC_ps", bufs=4, space="PSUM") as cps:
            oT_mine = omp.tile([P, KD * ROWS], f32, tag="oT_mine")
            xr = xrp.tile([P, KD * ROWS], f32, tag="xr")
            for kd in range(KD):
                # batch-0 rows (slots 0..3) then batch-1 rows (slots 4..7)
                nc.sync.dma_start(oT_mine[:, kd * ROWS:kd * ROWS + RH2].bitcast(f32r),
                                  oview[kd * P:(kd + 1) * P, :].bitcast(f32r))
                nc.sync.dma_start(oT_mine[:, kd * ROWS + RH2:(kd + 1) * ROWS].bitcast(f32r),
                                  oview[D + kd * P:D + (kd + 1) * P, :].bitcast(f32r))
                nc.sync.dma_start(xr[:, kd * ROWS:(kd + 1) * ROWS],
                                  xrT_in[kd * P:(kd + 1) * P, :])
            for mt in range(KD):
                wsb = wop.tile([P, KD * P], f32r, tag="wo")
                nc.sync.dma_start(wsb[:], wo_in[mt].rearrange("p k m -> p (k m)"))
                acc = cps.tile([P, ROWS], f32, tag="oacc")
                for kd in range(KD):
                    nc.tensor.matmul(
                        acc[:], wsb[:, kd * P:(kd + 1) * P],
                        oT_mine[:, kd * ROWS:(kd + 1) * ROWS].bitcast(f32r),
                        start=(kd == 0), stop=(kd == KD - 1))
                nc.vector.tensor_add(x1T[:, mt * ROWS:(mt + 1) * ROWS],
                                     acc[:], xr[:, mt * ROWS:(mt + 1) * ROWS])

        if dbg:
            nc.sync.dma_start(dbg["om"], oT_mine[:].bitcast(f32))
            nc.sync.dma_start(dbg["x1"], x1T[:])

        # norm2 + MLP
        mlp_stack = ExitStack()
        xn2p = mlp_stack.enter_context(tc.tile_pool(name="xn2", bufs=1))
        fnp = mlp_stack.enter_context(tc.tile_pool(name="fnT", bufs=1))
        xn2 = xn2p.tile([P, KD * ROWS], f32, tag="xn2")
        fnT = fnp.tile([P, FT * ROWS], bf16, tag="fnT")

        with tc.tile_pool(name="phD_sq", bufs=3) as sqp2, \
             tc.tile_pool(name="phD_r", bufs=2) as rp2, \
             tc.tile_pool(name="phD_ps", bufs=1, space="PSUM") as dps:
            ssq2 = dps.tile([1, ROWS], f32, tag="ssq2")
            for kd in range(KD):
                sq = sqp2.tile([P, ROWS], f32, tag="sq2")
                nc.scalar.activation(sq[:].bitcast(f32r), x1T[:, kd * ROWS:(kd + 1) * ROWS],
                                     mybir.ActivationFunctionType.Square)
                nc.tensor.matmul(ssq2[:], ones[:, 0:1],
                                 sq[:].bitcast(f32r),
                                 start=(kd == 0), stop=(kd == KD - 1))
            sqr2 = rp2.tile([1, ROWS], f32, tag="sqr2")
            nc.scalar.activation(sqr2[:], ssq2[:],
                                 mybir.ActivationFunctionType.Sqrt,
                                 bias=eps_sb[0:1, :], scale=1.0 / D)
            rr2 = rp2.tile([1, ROWS], f32, tag="rr2")
            with nc.allow_low_precision(reason="f32r rounding for PE broadcast"):
                nc.vector.reciprocal(rr2[:].bitcast(f32r), sqr2[:])
            rb2 = dps.tile([P, ROWS], f32, tag="rb2")
            nc.tensor.matmul(rb2[:], ones[0:1, :],
                             rr2[:].bitcast(f32r), start=True, stop=True)
            if dbg:
                dbg_rb2_sb = rp2.tile([P, ROWS], f32, tag="dbgrb2")
                nc.scalar.copy(dbg_rb2_sb[:], rb2[:])
                nc.sync.dma_start(dbg["rb2"], dbg_rb2_sb[:])
            for kd in range(KD):
                nc.vector.tensor_mul(xn2[:, kd * ROWS:(kd + 1) * ROWS].bitcast(f32r),
                                     x1T[:, kd * ROWS:(kd + 1) * ROWS], rb2[:])

        if dbg:
            nc.sync.dma_start(dbg["xn2"], xn2[:].bitcast(f32))

        with tc.tile_pool(name="phE_w", bufs=3) as wup_p, \
             tc.tile_pool(name="phE_sig", bufs=2) as sigp, \
             tc.tile_pool(name="phE_ps", bufs=4, space="PSUM") as eps_ps:
            for mt in range(FT):
                wsb = wup_p.tile([P, KD * P], f32r, tag="wup")
                nc.sync.dma_start(wsb[:], wup_in[mt].rearrange("p k m -> p (k m)"))
                acc = eps_ps.tile([P, ROWS], f32, tag="upacc")
                for kd in range(KD):
                    nc.tensor.matmul(
                        acc[:], wsb[:, kd * P:(kd + 1) * P],
                        xn2[:, kd * ROWS:(kd + 1) * ROWS].bitcast(f32r),
                        start=(kd == 0), stop=(kd == KD - 1))
                sig = sigp.tile([P, ROWS], f32, tag="sig")
                nc.scalar.activation(sig[:], acc[:],
                                     mybir.ActivationFunctionType.Sigmoid,
                                     bias=bup_sb[:, mt:mt + 1])
                # fn = (up + b_up) * sigmoid(up + b_up), cast to bf16
                nc.vector.scalar_tensor_tensor(
                    fnT[:, mt * ROWS:(mt + 1) * ROWS], acc[:],
                    bup_sb[:, mt:mt + 1], sig[:],
                    op0=mybir.AluOpType.add, op1=mybir.AluOpType.mult)
        if dbg:
            nc.sync.dma_start(dbg["fn"], fnT[:, 0:8 * ROWS])

        with tc.tile_pool(name="phF_w", bufs=2) as wdn_p, \
             tc.tile_pool(name="phF_out", bufs=2) as outp, \
             tc.tile_pool(name="phF_ps", bufs=4, space="PSUM") as fps:
            for mt in range(KD):
                wsb = wdn_p.tile([P, FT * P], bf16, tag="wdn")
                nc.sync.dma_start(wsb[:], wdn_in[mt].rearrange("p k m -> p (k m)"))
                acc = fps.tile([P, ROWS], f32, tag="dnacc")
                for kd in range(FT):
                    nc.tensor.matmul(
                        acc[:], wsb[:, kd * P:(kd + 1) * P],
                        fnT[:, kd * ROWS:(kd + 1) * ROWS],
                        start=(kd == 0), stop=(kd == FT - 1))
                out_sb = outp.tile([P, ROWS], f32, tag="out_sb")
                nc.vector.scalar_tensor_tensor(
                    out_sb[:], acc[:], bdn_sb[:, mt:mt + 1],
                    x1T[:, mt * ROWS:(mt + 1) * ROWS],
                    op0=mybir.AluOpType.add, op1=mybir.AluOpType.add)
                nc.sync.dma_start(outT[mt * P:(mt + 1) * P, :], out_sb[:])
        mlp_stack.close()
        x1_stack.close()


def host_prepare(inputs):
    """Fold LoRA/norm-weights/biases and build the 8 per-core input maps."""
    gi = {k: np.asarray(v, dtype=np.float32) if np.asarray(v).dtype != np.float32
          else np.asarray(v) for k, v in inputs.items()}

    def fold(nm):
        return gi['w_' + nm] + gi['w_' + nm + '_lora_a'] @ gi['w_' + nm + '_lora_b']

    nw1 = gi['norm_weight_1'][:, None]
    nw2 = gi['norm_weight_2'][:, None]
    w_q = (nw1 * fold('q')).astype(np.float32)
    w_k = (nw1 * fold('k')).astype(np.float32)
    w_v = (nw1 * fold('v')).astype(np.float32)
    w_o = fold('o').astype(np.float32)
    w_up = (nw2 * fold('up')).astype(np.float32)
    w_dn = fold('down').astype(np.float32)

    # pre-tiled weight layouts [mt, p, kd, m]
    wo_t = np.ascontiguousarray(
        w_o.reshape(KD, P, KD, P).transpose(2, 1, 0, 3))
    wup_t = np.ascontiguousarray(
        w_up.reshape(KD, P, FT, P).transpose(2, 1, 0, 3))
    wdn_t = np.ascontiguousarray(
        w_dn.reshape(FT, P, KD, P).transpose(2, 1, 0, 3)).astype(ml_dtypes.bfloat16)
    bup_t = np.ascontiguousarray(gi['b_up'].reshape(FT, P).T)
    bdn_t = np.ascontiguousarray(gi['b_down'].reshape(KD, P).T)

    cosT = np.ascontiguousarray(gi['cos'].T)
    sinTs = np.ascontiguousarray(gi['sin'].T)
    # rot(x).T = R @ x.T with R[d, d+64] = -1 (d<64), R[d, d-64] = +1;
    # matmul computes lhsT.T @ rhs, so pass R.T.
    Rm = np.zeros((P, P), dtype=np.float32)
    hh = HD // 2
    Rm[np.arange(hh), np.arange(hh) + hh] = -1.0
    Rm[np.arange(hh) + hh, np.arange(hh)] = 1.0
    rotmT = np.ascontiguousarray(Rm.T)
    maskT = np.maximum(gi['attention_mask'][0, 0, :P, :P], -2000.0).T
    wmask = np.full((P, 512), -2000.0, dtype=np.float32)
    wmask[:, 384:512] = maskT
    mask128 = np.ascontiguousarray(wmask)

    x = gi['x']
    b_o = gi['b_o']
    in_maps = []
    for i in range(N_CORES):
        b, g = divmod(i, 4)
        hs = slice(512 * g, 512 * (g + 1))
        wqkv = np.concatenate([w_q[:, hs], w_k[:, hs], w_v[:, hs]], axis=1)
        wqkv_t = np.ascontiguousarray(
            wqkv.reshape(KD, P, MQKV, P).transpose(2, 1, 0, 3))
        bqkv = np.concatenate([gi['b_q'][hs], gi['b_k'][hs], gi['b_v'][hs]])
        bqkv_t = np.ascontiguousarray(bqkv.reshape(MQKV, P).T)
        xbT = np.ascontiguousarray(x[b].T)
        # this core owns rows [256i, 256(i+1)) of BOTH batches
        xrows = np.concatenate(
            [x[0, RH2 * i:RH2 * (i + 1)], x[1, RH2 * i:RH2 * (i + 1)]], axis=0)
        xrT = np.ascontiguousarray(xrows.T + b_o[:, None])
        in_maps.append({
            "xbT": xbT, "xrT": xrT,
            "wqkv": wqkv_t, "bqkv": bqkv_t,
            "wo": wo_t, "wup": wup_t, "bup": bup_t,
            "wdn": wdn_t, "bdn": bdn_t,
            "cosT": cosT, "sinTs": sinTs, "rotmT": rotmT,
            "ones": np.ones((P, P), dtype=np.float32), "mask": mask128,
        })
    return in_maps


def assemble(results):
    out = np.empty((B, S, D), dtype=np.float32)
    for i in range(N_CORES):
        oT = results[i]["outT"]
        out[0, RH2 * i:RH2 * (i + 1), :] = oT[:, 0:RH2].T
        out[1, RH2 * i:RH2 * (i + 1), :] = oT[:, RH2:ROWS].T
    return out


_NC_CACHE = {}


def get_nc():
    if "nc" not in _NC_CACHE:
        _NC_CACHE["nc"] = build_program()
    return _NC_CACHE["nc"]


def kernel(**inputs):
    nc = get_nc()
    in_maps = host_prepare(inputs)
    res = run_bass_kernel_spmd(nc, in_maps, list(range(N_CORES)))
    return assemble(res.results)



# revision 15
# speedup vs baseline: 1.2657x; 1.2657x over previous
"""Trainium2 Bass kernel for nn_MixedSparseSingleLayer (dense transformer layer
with LoRA adapters): RMSNorm -> QKV(+LoRA) -> RoPE -> causal attention ->
O-proj(+LoRA) -> residual -> RMSNorm -> MLP silu(up)+down (+LoRA) -> residual.

Sharding (8 NeuronCores): 2-way data parallel over batch x 4-way tensor
parallel (Megatron). Within a 4-core batch group:
  - norm1 is replicated (cheap), QKV is column-sharded so each core owns 4
    attention heads end-to-end (RoPE + causal softmax + PV).
  - attention outputs are exchanged with four small per-head bf16 AllToAlls
    (pipelined behind later heads' attention) so each core then owns a
    512-row slice for row-parallel O-proj + MLP.

v3: all-bf16 PE path; QKV weights SBUF-resident; exp runs on 1024-wide
PSUM pairs for off-diagonal score blocks and sliced to the causal range on
diagonal blocks; softmax denominators accumulate on two wide DVE bf16 lanes
with one PE colsum per q-chunk and a pipelined division tail; causal mask is
a multiplicative 0/1 bf16 mask; reciprocals use the fast DVE approximation;
norm2 is fused into the O-projection loop; MLP up/down share one pool scope
so down-weights prefetch under the up-GEMM.
LoRA (rank 16), biases and RMSNorm weights are folded on the host (exact
algebraic rewrites); activations live in transposed [D, rows] layout so
matmuls run as lhsT=weight-tile / rhs=activation with N=512.
"""

import numpy as np
import ml_dtypes

import concourse.bass as bass
import concourse.mybir as mybir
import concourse.tile as tile
from concourse import bacc
from concourse.bass_utils import run_bass_kernel_spmd
from concourse.masks import make_identity

f32 = mybir.dt.float32
f32r = mybir.dt.float32r
bf16 = mybir.dt.bfloat16

B, S, D, H, HD, F, R = 2, 2048, 2048, 16, 128, 8192, 16
P = 128
KD = D // P            # 16 d_model tiles
MQKV = 12              # 1536/128 output tiles of the qkv shard
NH = 4                 # heads per core
QB = S // P            # 16 q blocks per head
FT = F // P            # 64
ROWS = 512             # rows owned per core (256 of each batch)
SCALE = 1.0 / float(np.sqrt(HD))
EPS = 1e-10

N_CORES = 8
GROUPS = [[0, 1, 2, 3, 4, 5, 6, 7]]
RH2 = ROWS // 2        # 256: rows owned per core per batch
NRH = 512              # rows per phase-A chunk


def build_program(single_core=False):
    nc = bacc.Bacc(
        "TRN2",
        target_bir_lowering=False,
        debug=False,
        num_devices=1 if single_core else N_CORES,
    )

    # ---- I/O ----
    xbT_in = nc.dram_tensor("xbT", [D, S], bf16, kind="ExternalInput").ap()
    xrT_in = nc.dram_tensor("xrT", [D, ROWS], bf16, kind="ExternalInput").ap()
    wqkv_in = nc.dram_tensor("wqkv", [MQKV, P, KD, P], bf16, kind="ExternalInput").ap()
    bqkv_in = nc.dram_tensor("bqkv", [P, MQKV], f32, kind="ExternalInput").ap()
    wo_in = nc.dram_tensor("wo", [KD, P, KD, P], bf16, kind="ExternalInput").ap()
    wup_in = nc.dram_tensor("wup", [FT, P, KD, P], bf16, kind="ExternalInput").ap()
    bup_in = nc.dram_tensor("bup", [P, FT], f32, kind="ExternalInput").ap()
    wdn_in = nc.dram_tensor("wdn", [KD, P, FT, P], bf16, kind="ExternalInput").ap()
    bdn_in = nc.dram_tensor("bdn", [P, KD], f32, kind="ExternalInput").ap()
    cosT_in = nc.dram_tensor("cosT", [P, S], bf16, kind="ExternalInput").ap()
    sinTs_in = nc.dram_tensor("sinTs", [P, S], bf16, kind="ExternalInput").ap()
    rotmT_in = nc.dram_tensor("rotmT", [P, P], bf16, kind="ExternalInput").ap()
    ones_in = nc.dram_tensor("ones", [P, P], bf16, kind="ExternalInput").ap()
    wm01_in = nc.dram_tensor("wm01", [P, 7 * P], bf16, kind="ExternalInput").ap()
    outT = nc.dram_tensor("outT", [D, ROWS], f32, kind="ExternalOutput").ap()

    with tile.TileContext(nc) as tc:
        _emit(tc, nc, xbT_in, xrT_in, wqkv_in, bqkv_in, wo_in, wup_in, bup_in,
              wdn_in, bdn_in, cosT_in, sinTs_in, rotmT_in, ones_in, wm01_in,
              outT, single_core)

    nc.compile()
    return nc


def _emit(tc, nc, xbT_in, xrT_in, wqkv_in, bqkv_in, wo_in, wup_in, bup_in,
          wdn_in, bdn_in, cosT_in, sinTs_in, rotmT_in, ones_in, wm01_in, outT,
          single_core=False):
    from contextlib import ExitStack

    top = ExitStack()
    with top:
        consts = top.enter_context(tc.tile_pool(name="consts", bufs=1))
        ident = consts.tile([P, P], bf16, tag="ident")
        make_identity(nc, ident[:])
        ones = consts.tile([P, P], bf16, tag="ones")
        nc.sync.dma_start(ones[:], ones_in)
        bqkv_sb = consts.tile([P, MQKV], f32, tag="bqkv")
        nc.sync.dma_start(bqkv_sb[:], bqkv_in)
        eps_sb = consts.tile([P, 1], f32, tag="eps")
        nc.vector.memset(eps_sb[:], EPS)
        # phase-B/E/F constants allocated now, DMAs deferred past the
        # phase-A critical path (wqkv + first x chunk first)
        wm01 = consts.tile([P, 7 * P], bf16, tag="wm01")
        cosT = consts.tile([P, S], bf16, tag="cosT")
        sinTs = consts.tile([P, S], bf16, tag="sinTs")
        rotmT = consts.tile([P, P], bf16, tag="rotmT")
        bup_sb = consts.tile([P, FT], f32, tag="bup")
        bdn_sb = consts.tile([P, KD], f32, tag="bdn")

        # DRAM staging for the per-head AllToAll exchange.
        dram = top.enter_context(tc.tile_pool(name="a2a", bufs=1, space="DRAM"))
        a2a_in = [dram.tile([N_CORES, P, RH2], bf16, tag=f"a2a_in{h}",
                            name=f"a2a_in{h}") for h in range(NH)]
        a2a_out = [dram.tile([N_CORES, P, RH2], bf16, tag=f"a2a_out{h}",
                             name=f"a2a_out{h}") for h in range(NH)]

        # ================= Phase A: norm1 (replicated) + QKV shard ==========
        qkv_stack = ExitStack()
        qkvp = qkv_stack.enter_context(tc.tile_pool(name="qkvT", bufs=1))
        qkvT = qkvp.tile([P, MQKV * S], bf16, tag="qkvT")

        with tc.tile_pool(name="phA_w", bufs=1) as wqp, \
             tc.tile_pool(name="phA_xb", bufs=2) as xbp, \
             tc.tile_pool(name="phA_xn", bufs=2) as xnp, \
             tc.tile_pool(name="phA_sq", bufs=2) as sqp, \
             tc.tile_pool(name="phA_r", bufs=2) as rp, \
             tc.tile_pool(name="phA_ps", bufs=4, space="PSUM") as pps, \
             tc.tile_pool(name="phA_st", bufs=1, space="PSUM") as stps:
            wqkv_sb = wqp.tile([P, MQKV * KD * P], bf16, tag="wqkv")
            # first x chunk + first weight tiles lead the DMA queues
            xb0 = xbp.tile([P, KD * NRH], bf16, tag="xb")
            for kd in range(KD):
                nc.sync.dma_start(
                    xb0[:, kd * NRH:(kd + 1) * NRH],
                    xbT_in[kd * P:(kd + 1) * P, 0:NRH])
            for mt in range(MQKV):
                nc.sync.dma_start(
                    wqkv_sb[:, mt * KD * P:(mt + 1) * KD * P],
                    wqkv_in[mt].rearrange("p k m -> p (k m)"))
            # deferred consts (needed from phase B on)
            nc.sync.dma_start(wm01[:], wm01_in)
            nc.sync.dma_start(cosT[:], cosT_in)
            nc.sync.dma_start(sinTs[:], sinTs_in)
            nc.sync.dma_start(rotmT[:], rotmT_in)
            nc.sync.dma_start(bup_sb[:], bup_in)
            nc.sync.dma_start(bdn_sb[:], bdn_in)

            for rh in range(S // NRH):
                if rh == 0:
                    xb = xb0
                else:
                    xb = xbp.tile([P, KD * NRH], bf16, tag="xb")
                    for kd in range(KD):
                        nc.sync.dma_start(
                            xb[:, kd * NRH:(kd + 1) * NRH],
                            xbT_in[kd * P:(kd + 1) * P, rh * NRH:(rh + 1) * NRH])
                # ssq[r] = sum_d x[d,r]^2: bf16 ACT squares summed on 4 DVE
                # lanes, then one PE ones-colsum.
                lanes = [None] * 4
                for kd in range(KD):
                    ln = kd % 4
                    if lanes[ln] is None:
                        sq = sqp.tile([P, NRH], bf16, tag=f"lane{ln}", bufs=2)
                        lanes[ln] = sq
                    else:
                        sq = sqp.tile([P, NRH], bf16, tag="sqt", bufs=3)
                    nc.scalar.activation(sq[:], xb[:, kd * NRH:(kd + 1) * NRH],
                                         mybir.ActivationFunctionType.Square)
                    if sq is not lanes[ln]:
                        nc.vector.tensor_add(lanes[ln][:], lanes[ln][:], sq[:])
                nc.vector.tensor_add(lanes[0][:], lanes[0][:], lanes[1][:])
                nc.vector.tensor_add(lanes[2][:], lanes[2][:], lanes[3][:])
                nc.vector.tensor_add(lanes[0][:], lanes[0][:], lanes[2][:])
                ssq = stps.tile([1, NRH], f32, tag="ssq")
                nc.tensor.matmul(ssq[:], ones[:, 0:1], lanes[0][:],
                                 start=True, stop=True)
                sqr = rp.tile([1, NRH], f32, tag="sqr")
                nc.scalar.activation(sqr[:], ssq[:],
                                     mybir.ActivationFunctionType.Sqrt,
                                     bias=eps_sb[0:1, :], scale=1.0 / D)
                rr = rp.tile([1, NRH], f32, tag="rr")
                nc.vector.reciprocal_approx_fast(out=rr[:], in_=sqr[:])
                rrb = rp.tile([1, NRH], bf16, tag="rrb")
                nc.vector.tensor_copy(rrb[:], rr[:])
                rb_ps = stps.tile([P, NRH], f32, tag="rb")
                nc.tensor.matmul(rb_ps[:], ones[0:1, :], rrb[:],
                                 start=True, stop=True)
                rb = rp.tile([P, NRH], bf16, tag="rbs")
                nc.vector.tensor_copy(rb[:], rb_ps[:])
                xn1 = xnp.tile([P, KD * NRH], bf16, tag="xn1")
                for kd in range(KD):
                    nc.vector.tensor_mul(xn1[:, kd * NRH:(kd + 1) * NRH],
                                         xb[:, kd * NRH:(kd + 1) * NRH], rb[:])
                # QKV matmuls: head-major m order so attention can start early
                for mt in (0, 4, 8, 1, 5, 9, 2, 6, 10, 3, 7, 11):
                    acc = pps.tile([P, NRH], f32, tag="qkvacc")
                    for kd in range(KD):
                        nc.tensor.matmul(
                            acc[:],
                            wqkv_sb[:, (mt * KD + kd) * P:(mt * KD + kd + 1) * P],
                            xn1[:, kd * NRH:(kd + 1) * NRH],
                            start=(kd == 0), stop=(kd == KD - 1))
                    nc.scalar.activation(
                        qkvT[:, mt * S + rh * NRH: mt * S + rh * NRH + NRH],
                        acc[:], mybir.ActivationFunctionType.Identity,
                        bias=bqkv_sb[:, mt:mt + 1])

        # ===== Phase C pools (right-side stack, opened early for prefetch) ==
        x1_stack = ExitStack()
        x1p = x1_stack.enter_context(
            tc.tile_pool(name="x1T", bufs=1, side="right"))
        x1T = x1p.tile([P, KD * ROWS], f32, tag="x1T")
        c_stack = ExitStack()
        omp = c_stack.enter_context(
            tc.tile_pool(name="phC_om", bufs=1, side="right"))
        xrp = c_stack.enter_context(
            tc.tile_pool(name="phC_xr", bufs=1, side="right"))
        wop = c_stack.enter_context(
            tc.tile_pool(name="phC_w", bufs=3, side="right"))
        oT_mine = omp.tile([P, KD * ROWS], bf16, tag="oT_mine")
        xr = xrp.tile([P, KD * ROWS], bf16, tag="xr")
        for kd in range(KD):
            nc.sync.dma_start(xr[:, kd * ROWS:(kd + 1) * ROWS],
                              xrT_in[kd * P:(kd + 1) * P, :])

        # ================= Phase B: attention (4 heads) =====================
        with tc.tile_pool(name="rope", bufs=2) as ropep, \
             tc.tile_pool(name="rtmp", bufs=2) as rtmpp, \
             tc.tile_pool(name="vnat", bufs=2) as vnatp, \
             tc.tile_pool(name="prw", bufs=3) as prwp, \
             tc.tile_pool(name="prd", bufs=3) as prdp, \
             tc.tile_pool(name="lanew", bufs=2) as lwp, \
             tc.tile_pool(name="lsml", bufs=2) as lp, \
             tc.tile_pool(name="oT", bufs=2) as otp, \
             tc.tile_pool(name="sc_ps", bufs=2, space="PSUM") as scps, \
             tc.tile_pool(name="tr_ps", bufs=1, space="PSUM") as trps, \
             tc.tile_pool(name="ov_ps", bufs=2, space="PSUM") as ovps, \
             tc.tile_pool(name="st_ps", bufs=1, space="PSUM") as stp2:
            for h in range(NH):
                q_sl = qkvT[:, h * S:(h + 1) * S]
                k_sl = qkvT[:, (NH + h) * S:(NH + h + 1) * S]
                v_sl = qkvT[:, (2 * NH + h) * S:(2 * NH + h + 1) * S]
                # RoPE on q and k (rotate_half via PE signed-permutation mm)
                rq = ropep.tile([P, S], bf16, tag="ropeq")
                rk = ropep.tile([P, S], bf16, tag="ropek")
                for src, dst in ((q_sl, rq), (k_sl, rk)):
                    for c in range(S // NRH):
                        sl = slice(c * NRH, (c + 1) * NRH)
                        rtw = scps.tile([P, 2 * NRH], f32, tag="sccw")
                        rt = rtw[:, 0:NRH]
                        nc.tensor.matmul(rt, rotmT[:], src[:, sl],
                                         start=True, stop=True)
                        tmp = rtmpp.tile([P, NRH], bf16, tag="rtmp")
                        nc.vector.tensor_mul(tmp[:], rt, sinTs[:, sl])
                        nc.vector.tensor_mul(dst[:, sl], src[:, sl], cosT[:, sl])
                        nc.vector.tensor_add(dst[:, sl], dst[:, sl], tmp[:])
                # v in natural layout [S, HD] (PE transpose per 128-block)
                vnat = vnatp.tile([P, QB * P], bf16, tag="vnat")
                for kt in range(QB):
                    tp = trps.tile([P, P], bf16, tag="trp")
                    nc.tensor.transpose(tp[:], v_sl[:, kt * P:(kt + 1) * P],
                                        ident[:])
                    nc.vector.tensor_copy(vnat[:, kt * P:(kt + 1) * P], tp[:])
                oTh = otp.tile([P, S], bf16, tag="oTh")
                # scores TRANSPOSED (s.T[S_k, q]); off-diagonal blocks in
                # 1024-wide pairs (one exp per pair); diagonal blocks sliced
                # to the causal range.  Softmax division tail of chunk qc is
                # emitted during chunk qc+1 to keep the PE queue busy.
                pend = None  # (opsum, lps, qc)

                def div_tail(pend):
                    opsum_, stat_, qc_ = pend
                    rinv = lp.tile([1, NRH], f32, tag="rinv")
                    nc.vector.reciprocal_approx_fast(out=rinv[:],
                                                     in_=stat_[0:1, :])
                    rinvb = lp.tile([1, NRH], bf16, tag="rinvb")
                    nc.vector.tensor_copy(rinvb[:], rinv[:])
                    nc.tensor.matmul(stat_[:], ones[0:1, :], rinvb[:],
                                     start=True, stop=True)
                    rbs = lp.tile([P, NRH], bf16, tag="rbs")
                    nc.vector.tensor_copy(rbs[:], stat_[:])
                    nc.vector.tensor_mul(oTh[:, qc_ * NRH:(qc_ + 1) * NRH],
                                         opsum_[:], rbs[:])

                for qc in range(S // NRH):
                    opsum = ovps.tile([P, NRH], f32, tag="opv")
                    lanes = [None, None]   # two [P, 2*NRH] bf16 lanes
                    npair = 2 * qc         # off-diagonal pairs
                    for pr_i in range(npair):
                        kt0 = 2 * pr_i
                        sccw = scps.tile([P, 2 * NRH], f32, tag="sccw")
                        for half in (0, 1):
                            nc.tensor.matmul(
                                sccw[:, half * NRH:(half + 1) * NRH],
                                rk[:, (kt0 + half) * P:(kt0 + half + 1) * P],
                                rq[:, qc * NRH:(qc + 1) * NRH],
                                start=True, stop=True)
                        ln = pr_i % 2
                        if lanes[ln] is None:
                            prw = lwp.tile([P, 2 * NRH], bf16, tag=f"lane{ln}")
                            lanes[ln] = prw
                            fresh = True
                        else:
                            prw = prwp.tile([P, 2 * NRH], bf16, tag="prw")
                            fresh = False
                        nc.scalar.activation(
                            prw[:], sccw[:],
                            mybir.ActivationFunctionType.Exp, scale=SCALE)
                        for half in (0, 1):
                            nc.tensor.matmul(
                                opsum[:],
                                vnat[:, (kt0 + half) * P:(kt0 + half + 1) * P],
                                prw[:, half * NRH:(half + 1) * NRH],
                                start=(kt0 + half == 0), stop=False)
                        if not fresh:
                            nc.vector.tensor_add(lanes[ln][:], lanes[ln][:],
                                                 prw[:])
                        if pr_i == 0 and pend is not None:
                            div_tail(pend)
                            pend = None
                    # diagonal blocks lb = 0..3 (kt = 4qc+lb), causal-sliced
                    for lb in range(4):
                        kt = 4 * qc + lb
                        q0 = lb * P
                        if qc == 0:
                            # exp writes straight into a lane half; zero the
                            # lane first so untouched columns stay exact 0
                            ln, half = lb % 2, lb // 2
                            if lanes[ln] is None:
                                lanes[ln] = lwp.tile([P, 2 * NRH], bf16,
                                                     tag=f"lane{ln}",
                                                     name=f"lanez{ln}")
                                nc.vector.memset(lanes[ln][:], 0.0)
                            prd = lanes[ln][:, half * NRH:(half + 1) * NRH]
                            add_after = False
                        else:
                            prdt = prdp.tile([P, NRH], bf16, tag="prd")
                            prd = prdt[:]
                            if q0:
                                nc.vector.memset(prd[:, 0:q0], 0.0)
                            add_after = True
                        sccw = scps.tile([P, 2 * NRH], f32, tag="sccw")
                        nc.tensor.matmul(
                            sccw[:, q0:NRH],
                            rk[:, kt * P:(kt + 1) * P],
                            rq[:, qc * NRH + q0:(qc + 1) * NRH],
                            start=True, stop=True)
                        nc.scalar.activation(
                            prd[:, q0:NRH], sccw[:, q0:NRH],
                            mybir.ActivationFunctionType.Exp, scale=SCALE)
                        # in-block triangular causal mask (multiplicative)
                        nc.vector.tensor_mul(prd[:, q0:q0 + P],
                                             prd[:, q0:q0 + P],
                                             wm01[:, 3 * P:4 * P])
                        nc.tensor.matmul(
                            opsum[:], vnat[:, kt * P:(kt + 1) * P], prd,
                            start=(kt == 0), stop=(lb == 3))
                        if add_after:
                            ln, half = lb % 2, lb // 2
                            nc.vector.tensor_add(
                                lanes[ln][:, half * NRH + q0:(half + 1) * NRH],
                                lanes[ln][:, half * NRH + q0:(half + 1) * NRH],
                                prd[:, q0:NRH])
                        if qc == 0 and lb == 1 and pend is not None:
                            div_tail(pend)
                            pend = None
                    # fold lanes: lane0 += lane1 (wide), then halves
                    nc.vector.tensor_add(lanes[0][:], lanes[0][:], lanes[1][:])
                    nc.vector.tensor_add(lanes[0][:, 0:NRH], lanes[0][:, 0:NRH],
                                         lanes[0][:, NRH:2 * NRH])
                    stat = stp2.tile([P, NRH], f32, tag="stat")
                    nc.tensor.matmul(stat[0:1, :], ones[:, 0:1],
                                     lanes[0][:, 0:NRH], start=True, stop=True)
                    pend = (opsum, stat, qc)
                div_tail(pend)
                pend = None
                # ship this head's output into its AllToAll staging buffer
                for j in range(N_CORES):
                    nc.sync.dma_start(a2a_in[h][j],
                                      oTh[:, j * RH2:(j + 1) * RH2])
                if single_core:
                    nc.sync.dma_start(
                        a2a_out[h][:].rearrange("a r c -> (a r) c"),
                        a2a_in[h][:].rearrange("a r c -> (a r) c"))
                else:
                    nc.gpsimd.collective_compute(
                        "AllToAll", mybir.AluOpType.bypass,
                        replica_groups=GROUPS,
                        ins=[a2a_in[h][:].opt()],
                        outs=[a2a_out[h][:].opt()],
                    )
                # receive: src core j, local head h -> global d block
                # 4*(j%4)+h, batch half j//4 (lands during head h+1 compute)
                for j in range(N_CORES):
                    kdb = 4 * (j % 4) + h
                    half = j // 4
                    nc.sync.dma_start(
                        oT_mine[:, kdb * ROWS + half * RH2:
                                kdb * ROWS + (half + 1) * RH2],
                        a2a_out[h][j])
        qkv_stack.close()

        # ============ Phase C+D: row-parallel O-proj with fused norm2 =======
        mlp_stack = ExitStack()
        xn2p = mlp_stack.enter_context(tc.tile_pool(name="xn2", bufs=1))
        fnp = mlp_stack.enter_context(tc.tile_pool(name="fnT", bufs=1))
        xn2 = xn2p.tile([P, KD * ROWS], bf16, tag="xn2")
        fnT = fnp.tile([P, FT * ROWS], bf16, tag="fnT")

        with tc.tile_pool(name="phD_sq", bufs=2) as sqp2, \
             tc.tile_pool(name="phD_r", bufs=2) as rp2, \
             tc.tile_pool(name="phC_ps", bufs=4, space="PSUM") as cps, \
             tc.tile_pool(name="phD_ps", bufs=1, space="PSUM") as dps:
            lanes2 = [None] * 4
            for mt in range(KD):
                wsb = wop.tile([P, KD * P], bf16, tag="wo")
                nc.sync.dma_start(wsb[:], wo_in[mt].rearrange("p k m -> p (k m)"))
                acc = cps.tile([P, ROWS], f32, tag="oacc")
                for kd in range(KD):
                    nc.tensor.matmul(
                        acc[:], wsb[:, kd * P:(kd + 1) * P],
                        oT_mine[:, kd * ROWS:(kd + 1) * ROWS],
                        start=(kd == 0), stop=(kd == KD - 1))
                x1sl = x1T[:, mt * ROWS:(mt + 1) * ROWS]
                nc.vector.tensor_add(x1sl, acc[:],
                                     xr[:, mt * ROWS:(mt + 1) * ROWS])
                # norm2 statistics, fused per-mt
                ln = mt % 4
                if lanes2[ln] is None:
                    sq = sqp2.tile([P, ROWS], bf16, tag=f"lane{ln}", bufs=1)
                    lanes2[ln] = sq
                else:
                    sq = sqp2.tile([P, ROWS], bf16, tag="sqt", bufs=3)
                nc.scalar.activation(sq[:], x1sl,
                                     mybir.ActivationFunctionType.Square)
                if sq is not lanes2[ln]:
                    nc.vector.tensor_add(lanes2[ln][:], lanes2[ln][:], sq[:])
            nc.vector.tensor_add(lanes2[0][:], lanes2[0][:], lanes2[1][:])
            nc.vector.tensor_add(lanes2[2][:], lanes2[2][:], lanes2[3][:])
            nc.vector.tensor_add(lanes2[0][:], lanes2[0][:], lanes2[2][:])
            ssq2 = dps.tile([1, ROWS], f32, tag="ssq2")
            nc.tensor.matmul(ssq2[:], ones[:, 0:1], lanes2[0][:],
                             start=True, stop=True)
            sqr2 = rp2.tile([1, ROWS], f32, tag="sqr2")
            nc.scalar.activation(sqr2[:], ssq2[:],
                                 mybir.ActivationFunctionType.Sqrt,
                                 bias=eps_sb[0:1, :], scale=1.0 / D)
            rr2 = rp2.tile([1, ROWS], f32, tag="rr2")
            nc.vector.reciprocal_approx_fast(out=rr2[:], in_=sqr2[:])
            rr2b = rp2.tile([1, ROWS], bf16, tag="rr2b")
            nc.vector.tensor_copy(rr2b[:], rr2[:])
            rb2_ps = dps.tile([P, ROWS], f32, tag="rb2")
            nc.tensor.matmul(rb2_ps[:], ones[0:1, :], rr2b[:],
                             start=True, stop=True)
            rb2 = rp2.tile([P, ROWS], bf16, tag="rb2s")
            nc.vector.tensor_copy(rb2[:], rb2_ps[:])
            for kd in range(KD):
                nc.vector.tensor_mul(xn2[:, kd * ROWS:(kd + 1) * ROWS],
                                     x1T[:, kd * ROWS:(kd + 1) * ROWS], rb2[:])
        # release O-proj inputs (oT_mine, xr, wo); x1T stays for phase F
        c_stack.close()

        # ================= Phase E+F: MLP up+silu, down+residual ============
        with tc.tile_pool(name="phE_w", bufs=4) as wup_p, \
             tc.tile_pool(name="phE_sig", bufs=3) as sigp, \
             tc.tile_pool(name="phF_w", bufs=2) as wdn_p, \
             tc.tile_pool(name="phF_out", bufs=2) as outp, \
             tc.tile_pool(name="phEF_ps", bufs=4, space="PSUM") as eps_ps:
            for mt in range(FT):
                wsb = wup_p.tile([P, KD * P], bf16, tag="wup")
                nc.sync.dma_start(wsb[:], wup_in[mt].rearrange("p k m -> p (k m)"))
                acc = eps_ps.tile([P, ROWS], f32, tag="upacc")
                for kd in range(KD):
                    nc.tensor.matmul(
                        acc[:], wsb[:, kd * P:(kd + 1) * P],
                        xn2[:, kd * ROWS:(kd + 1) * ROWS],
                        start=(kd == 0), stop=(kd == KD - 1))
                sig = sigp.tile([P, ROWS], bf16, tag="sig")
                nc.scalar.activation(sig[:], acc[:],
                                     mybir.ActivationFunctionType.Sigmoid,
                                     bias=bup_sb[:, mt:mt + 1])
                nc.vector.scalar_tensor_tensor(
                    fnT[:, mt * ROWS:(mt + 1) * ROWS], acc[:],
                    bup_sb[:, mt:mt + 1], sig[:],
                    op0=mybir.AluOpType.add, op1=mybir.AluOpType.mult)
            for mt in range(KD):
                wsb = wdn_p.tile([P, FT * P], bf16, tag="wdn")
                nc.sync.dma_start(wsb[:], wdn_in[mt].rearrange("p k m -> p (k m)"))
                acc = eps_ps.tile([P, ROWS], f32, tag="dnacc")
                for kd in range(FT):
                    nc.tensor.matmul(
                        acc[:], wsb[:, kd * P:(kd + 1) * P],
                        fnT[:, kd * ROWS:(kd + 1) * ROWS],
                        start=(kd == 0), stop=(kd == FT - 1))
                out_sb = outp.tile([P, ROWS], f32, tag="out_sb")
                nc.vector.scalar_tensor_tensor(
                    out_sb[:], acc[:], bdn_sb[:, mt:mt + 1],
                    x1T[:, mt * ROWS:(mt + 1) * ROWS],
                    op0=mybir.AluOpType.add, op1=mybir.AluOpType.add)
                nc.sync.dma_start(outT[mt * P:(mt + 1) * P, :], out_sb[:])
        mlp_stack.close()
        x1_stack.close()


def host_prepare(inputs):
    """Fold LoRA/norm-weights/biases and build the 8 per-core input maps."""
    gi = {k: np.asarray(v, dtype=np.float32) if np.asarray(v).dtype != np.float32
          else np.asarray(v) for k, v in inputs.items()}
    b16 = ml_dtypes.bfloat16

    def fold(nm):
        return gi['w_' + nm] + gi['w_' + nm + '_lora_a'] @ gi['w_' + nm + '_lora_b']

    nw1 = gi['norm_weight_1'][:, None]
    nw2 = gi['norm_weight_2'][:, None]
    w_q = (nw1 * fold('q')).astype(np.float32)
    w_k = (nw1 * fold('k')).astype(np.float32)
    w_v = (nw1 * fold('v')).astype(np.float32)
    w_o = fold('o').astype(np.float32)
    w_up = (nw2 * fold('up')).astype(np.float32)
    w_dn = fold('down').astype(np.float32)

    wo_t = np.ascontiguousarray(
        w_o.reshape(KD, P, KD, P).transpose(2, 1, 0, 3)).astype(b16)
    wup_t = np.ascontiguousarray(
        w_up.reshape(KD, P, FT, P).transpose(2, 1, 0, 3)).astype(b16)
    wdn_t = np.ascontiguousarray(
        w_dn.reshape(FT, P, KD, P).transpose(2, 1, 0, 3)).astype(b16)
    bup_t = np.ascontiguousarray(gi['b_up'].reshape(FT, P).T)
    bdn_t = np.ascontiguousarray(gi['b_down'].reshape(KD, P).T)

    cosT = np.ascontiguousarray(gi['cos'].T).astype(b16)
    sinTs = np.ascontiguousarray(gi['sin'].T).astype(b16)
    Rm = np.zeros((P, P), dtype=np.float32)
    hh = HD // 2
    Rm[np.arange(hh), np.arange(hh) + hh] = -1.0
    Rm[np.arange(hh) + hh, np.arange(hh)] = 1.0
    rotmT = np.ascontiguousarray(Rm.T).astype(b16)
    # multiplicative causal mask bank [P, 7*128] bf16:
    # cols 0..384 zero, 384..512 in-block lower-triangular, 512..896 one.
    tri = (np.arange(P)[:, None] <= np.arange(P)[None, :]).astype(np.float32)
    wm01 = np.concatenate(
        [np.zeros((P, 3 * P), np.float32), tri, np.ones((P, 3 * P), np.float32)],
        axis=1).astype(b16)

    x = gi['x']
    b_o = gi['b_o']
    in_maps = []
    for i in range(N_CORES):
        b, g = divmod(i, 4)
        hs = slice(512 * g, 512 * (g + 1))
        wqkv = np.concatenate([w_q[:, hs], w_k[:, hs], w_v[:, hs]], axis=1)
        wqkv_t = np.ascontiguousarray(
            wqkv.reshape(KD, P, MQKV, P).transpose(2, 1, 0, 3)).astype(b16)
        bqkv = np.concatenate([gi['b_q'][hs], gi['b_k'][hs], gi['b_v'][hs]])
        bqkv_t = np.ascontiguousarray(bqkv.reshape(MQKV, P).T)
        xbT = np.ascontiguousarray(x[b].T).astype(b16)
        xrows = np.concatenate(
            [x[0, RH2 * i:RH2 * (i + 1)], x[1, RH2 * i:RH2 * (i + 1)]], axis=0)
        xrT = np.ascontiguousarray(xrows.T + b_o[:, None]).astype(b16)
        in_maps.append({
            "xbT": xbT, "xrT": xrT,
            "wqkv": wqkv_t, "bqkv": bqkv_t,
            "wo": wo_t, "wup": wup_t, "bup": bup_t,
            "wdn": wdn_t, "bdn": bdn_t,
            "cosT": cosT, "sinTs": sinTs, "rotmT": rotmT,
            "ones": np.ones((P, P), dtype=b16), "wm01": wm01,
        })
    return in_maps


def assemble(results):
    out = np.empty((B, S, D), dtype=np.float32)
    for i in range(N_CORES):
        oT = results[i]["outT"]
        out[0, RH2 * i:RH2 * (i + 1), :] = oT[:, 0:RH2].T
        out[1, RH2 * i:RH2 * (i + 1), :] = oT[:, RH2:ROWS].T
    return out


_NC_CACHE = {}


def get_nc():
    if "nc" not in _NC_CACHE:
        _NC_CACHE["nc"] = build_program()
    return _NC_CACHE["nc"]


def kernel(**inputs):
    nc = get_nc()
    in_maps = host_prepare(inputs)
    res = run_bass_kernel_spmd(nc, in_maps, list(range(N_CORES)))
    return assemble(res.results)
